# revision 22
# baseline (speedup 1.0000x reference)
"""Trainium2 Bass kernel for nn_AttentionMask_13048110645633.

Math: for key (4,32,64,64) and query (4,512), with s = key.reshape(B,J)
and q = query, the reference computes per element

    ctx[b,j] = sum_k q[b,k]*exp(s[b,j]*q[b,k]) / sum_k exp(s[b,j]*q[b,k])
    out[b,j] = s[b,j] * sigmoid(ctx[b,j])

i.e. out = s * g_b(s) where g_b is a smooth scalar function determined by
q[b].  Sharding: data-parallel over B (4 batches x 2 half-slabs = 8 cores).

Fast variant, per core (one (128,512) tile = half a batch):
  1. broadcast q to all partitions with a C=1 PE matmul (no 256KB DMA),
  2. evaluate g_b exactly at 128 fixed Chebyshev nodes: ACT exp with
     per-partition scale + fused accumulate, one fused DVE multiply-reduce,
     reciprocal, tanh -> 128 gate values,
  3. contract with a constant least-squares matrix on the PE to get the
     Chebyshev coefficients of g_b in t = tanh(a*s), replicated on all
     partitions,
  4. evaluate with a Clenshaw recurrence on the DVE over the whole tile
     (high-order steps in fp16 at 2 elem/cycle, low-order in fp32),
  5. out = s * g.

The direct variant brute-forces the (J,K) slab; used for cross-checking.
"""

import os
import numpy as np

B, J, K = 4, 131072, 512
P, F = 128, 512  # per-core tile (P*F = J/2)
NCORES = 8
D = 18          # Chebyshev degree
K0 = 6          # steps k >= K0 run in fp16, k < K0 in fp32
NNODES = 128
WARP_A = 0.35
SRANGE = 5.5

_CONSTS = None
_NC_CACHE = {}


def _host_constants():
    """Data-independent fit constants (nodes, sign-folded fit matrix)."""
    global _CONSTS
    if _CONSTS is not None:
        return _CONSTS
    import numpy.polynomial.chebyshev as _C

    tmax = float(np.tanh(WARP_A * SRANGE))
    th = (np.arange(NNODES) + 0.5) * np.pi / NNODES
    un = np.cos(th)  # Chebyshev points in [-1,1]
    sn = np.arctanh(un * tmax) / WARP_A  # node s-values
    V = _C.chebvander(un, D)  # (N, D+1)
    G = np.linalg.pinv(V)  # (D+1, N): node values -> cheb coeffs

    # Clenshaw sign schedule: A_k = eps_k * b_k with eps_k = -eps_{k+2} so
    # that each step is one scalar_tensor_tensor: A_k = (A_{k+2} + eps_k*c_k)
    # op1 (2u (*) A_{k+1}).  Require eps_2 = -1 for the final step.
    eps = {}
    for chain in (list(range(2, D + 1, 2))[::-1], list(range(1, D + 1, 2))[::-1]):
        n = len(chain)
        top = (-1) ** n if chain[-1] == 2 else 1
        for i, k in enumerate(chain):
            eps[k] = top * ((-1) ** i)
    sigma = np.array([1] + [eps[k] for k in range(1, D + 1)], dtype=np.float64)
    Gs = G * sigma[:, None]
    gt = np.ascontiguousarray(Gs.T.astype(np.float32))  # (N, D+1)
    _CONSTS = (tmax, sn.astype(np.float32).reshape(NNODES, 1), gt, eps)
    return _CONSTS


def _mock_core(s_tile, qb_tile, sn, gt, tmax, eps):
    """Pure-numpy f32/fp16 mirror of the fast device program (debugging)."""
    f = np.float32
    h = np.float16
    En = np.exp(qb_tile * sn).astype(f)
    S0n = En.sum(1, dtype=f).reshape(-1, 1)
    S1n = (En * qb_tile).sum(1, dtype=f).reshape(-1, 1)
    ctxn = (S1n * (f(1) / S0n)).astype(f)
    gaten = (f(0.5) * np.tanh(f(0.5) * ctxn) + f(0.5)).astype(f)
    c = (gaten[:, 0].astype(f) @ gt).astype(f)  # (D+1,)
    T = np.tanh(f(WARP_A) * s_tile).astype(f)
    U2 = (T * f(2.0 / tmax)).astype(f)
    U1 = (T * f(eps[1] / tmax)).astype(f)
    U2h = U2.astype(h)
    A = {D + 1: np.zeros(s_tile.shape, h), D: np.full(s_tile.shape, c[D], dtype=h)}
    for k in range(D - 1, 0, -1):
        if k >= K0:
            tmp = (U2h.astype(f) * A[k + 1].astype(f)).astype(h)
            x = (A[k + 2].astype(f) + c[k]).astype(h)
            y = x.astype(f) + tmp.astype(f) if eps[k] * eps[k + 1] == 1 \
                else x.astype(f) - tmp.astype(f)
            A[k] = y.astype(h)
        else:
            a1 = A[k + 1].astype(f)
            a2 = A[k + 2].astype(f)
            tmp = (U2 * a1).astype(f)
            x = (a2 + c[k]).astype(f)
            A[k] = (x + tmp).astype(f) if eps[k] * eps[k + 1] == 1 \
                else (x - tmp).astype(f)
    tmpf = (U1 * A[1].astype(f)).astype(f)
    g = ((A[2].astype(f) + c[0]).astype(f) + tmpf).astype(f)
    return (s_tile * g).astype(f)


def _build_nc(variant):
    import concourse.bacc as bacc
    import concourse.mybir as mybir
    from concourse import tile

    fp32 = mybir.dt.float32
    fp16 = mybir.dt.float16
    AF = mybir.ActivationFunctionType
    OP = mybir.AluOpType
    tmax, _sn, _gt, eps = _host_constants()

    nc = bacc.Bacc("TRN2", target_bir_lowering=False, debug=False, num_devices=NCORES)
    s_d = nc.dram_tensor("s", (P, F), fp32, kind="ExternalInput")
    qp_d = nc.dram_tensor("qpair", (2, K), fp16, kind="ExternalInput")
    sn_d = nc.dram_tensor("sn", (NNODES, 1), fp32, kind="ExternalInput")
    gt_d = nc.dram_tensor("gt", (NNODES, D + 1), fp32, kind="ExternalInput")
    y_d = nc.dram_tensor("y", (P, F), fp32, kind="ExternalOutput")

    with tile.TileContext(nc) as tc:
        with (
            tc.tile_pool(name="c1", bufs=1) as cp,
            tc.tile_pool(name="ab", bufs=K0 + 2) as ab,
            tc.tile_pool(name="abh", bufs=D - K0 + 2) as abh,
            tc.tile_pool(name="xh", bufs=D - K0 + 2) as xh,
            tc.tile_pool(name="tm", bufs=K0 + 2) as tp,
            tc.tile_pool(name="tmh", bufs=D - K0 + 2) as tph,
            tc.tile_pool(name="wk", bufs=3) as wp,
            tc.tile_pool(name="ps", bufs=1, space="PSUM") as pp,
        ):
            # hoist the ~1.3us activation-table load into the DMA window: a
            # 1-element activation whose only dep is a local memset makes
            # walrus place PSEUDO_LOAD_ACT_FUNC_SET at t~0
            zz = cp.tile([1, 1], fp32, tag="zz")
            nc.gpsimd.memset(zz[:], 0.0)
            zz2 = cp.tile([1, 1], fp32, tag="zz2")
            nc.scalar.activation(zz2[:], zz[:], AF.Exp)

            # q first (the fit pipeline hangs off it), then sn, then the bulk
            qp_sb = cp.tile([2, K], fp16, tag="qp_sb")
            nc.sync.dma_start(out=qp_sb[:], in_=qp_d[:])
            snt = cp.tile([NNODES, 1], fp32, tag="snt")
            nc.sync.dma_start(out=snt[:], in_=sn_d[:])
            s_all = cp.tile([P, F], fp32, tag="s_all")
            nc.sync.dma_start(out=s_all[:], in_=s_d[:])
            gtt = cp.tile([NNODES, D + 1], fp32, tag="gtt")
            nc.sync.dma_start(out=gtt[:], in_=gt_d[:])

            # broadcast q to all 128 partitions exactly: q = qhi + qlo as an
            # fp16 pair summed by a single C=2 matmul into fp32 PSUM
            ones = cp.tile([2, P], fp16, tag="ones")
            nc.gpsimd.memset(ones[:], 1.0)
            q_ps = pp.tile([P, K], fp32, tag="qps")
            nc.tensor.matmul(q_ps[:], ones[:], qp_sb[:], start=True, stop=True)

            if variant == "fast":
                # ---- evaluate g at the fixed nodes (one partition each)
                En = cp.tile([NNODES, K], fp32, tag="En")
                S0n = cp.tile([NNODES, 1], fp32, tag="S0n")
                nc.scalar.activation(
                    En[:], q_ps[:], AF.Exp, scale=snt[:], accum_out=S0n[:]
                )
                # warp for the main tile, emitted early so ACT runs it right
                # after the node exp while the DVE digests the node sums
                T = cp.tile([P, F], fp32, tag="T")
                nc.scalar.activation(T[:], s_all[:], AF.Tanh, scale=float(WARP_A))
                scrn = cp.tile([NNODES, K], fp32, tag="scrn")
                S1n = cp.tile([NNODES, 1], fp32, tag="S1n")
                nc.vector.scalar_tensor_tensor(
                    out=scrn[:], in0=En[:], scalar=1.0, in1=q_ps[:],
                    op0=OP.mult, op1=OP.mult, accum_out=S1n[:],
                )
                recn = cp.tile([NNODES, 1], fp32, tag="recn")
                nc.vector.reciprocal(recn[:], S0n[:])
                ctxn = cp.tile([NNODES, 1], fp32, tag="ctxn")
                nc.vector.tensor_tensor(ctxn[:], S1n[:], recn[:], OP.mult)
                thn = cp.tile([NNODES, 1], fp32, tag="thn")
                nc.scalar.activation(thn[:], ctxn[:], AF.Tanh, scale=0.5)
                gaten = cp.tile([NNODES, 1], fp32, tag="gaten")
                nc.vector.tensor_scalar(
                    out=gaten[:], in0=thn[:], scalar1=0.5, scalar2=0.5,
                    op0=OP.mult, op1=OP.add,
                )
                # broadcast gate along free dim, then PE-contract with gt to
                # land the coefficients replicated on all 128 partitions
                gbf = cp.tile([P, P], fp32, tag="gbf")
                nc.vector.tensor_scalar(
                    out=gbf[:], in0=En[:, 0:P], scalar1=0.0, scalar2=gaten[:],
                    op0=OP.mult, op1=OP.add,
                )
                c_ps = pp.tile([P, D + 1], fp32, tag="cps")
                nc.tensor.matmul(c_ps[:], gbf[:], gtt[:], start=True, stop=True)
                c_sb = cp.tile([P, D + 1], fp32, tag="csb")
                nc.vector.tensor_copy(c_sb[:], c_ps[:])

                # ---- main evaluation over the whole (P,F) tile
                U2 = cp.tile([P, F], fp32, tag="U2")
                nc.vector.tensor_scalar(
                    out=U2[:], in0=T[:], scalar1=float(2.0 / tmax), scalar2=None,
                    op0=OP.mult,
                )
                U2h = cp.tile([P, F], fp16, tag="U2h")
                nc.vector.tensor_copy(U2h[:], U2[:])

                A = {}
                aD = abh.tile([P, F], fp16, tag="Ah")
                nc.vector.tensor_scalar(
                    out=aD[:], in0=En[:], scalar1=0.0, scalar2=c_sb[:, D:D + 1],
                    op0=OP.mult, op1=OP.add,
                )
                A[D] = aD
                h_impl = os.environ.get("BASS_FP16_STEP", "act")
                for k in range(D - 1, 0, -1):
                    op_add = eps[k] * eps[k + 1] == 1
                    if k >= K0:
                        tmp = tph.tile([P, F], fp16, tag="tmph")
                        nc.vector.tensor_tensor(tmp[:], U2h[:], A[k + 1][:], OP.mult)
                        ak = abh.tile([P, F], fp16, tag="Ah")
                        if k == D - 1:
                            # A_{k+2} is identically zero: x = c_k broadcast,
                            # cheap 4x-mode ts on the DVE
                            x = xh.tile([P, F], fp16, tag="X")
                            nc.vector.tensor_scalar(
                                out=x[:], in0=En[:], scalar1=0.0,
                                scalar2=c_sb[:, k:k + 1], op0=OP.mult, op1=OP.add,
                            )
                            nc.vector.tensor_tensor(
                                ak[:], x[:], tmp[:],
                                OP.add if op_add else OP.subtract,
                            )
                        elif h_impl == "act":
                            # the "+c_k" runs on the otherwise idle ACT engine
                            # (2 steps of slack), DVE does two 2x-mode tts
                            x = xh.tile([P, F], fp16, tag="X")
                            nc.scalar.activation(
                                x[:], A[k + 2][:], AF.Identity,
                                bias=c_sb[:, k:k + 1], scale=1.0,
                            )
                            nc.vector.tensor_tensor(
                                ak[:], x[:], tmp[:],
                                OP.add if op_add else OP.subtract,
                            )
                        elif h_impl == "ts":
                            x = xh.tile([P, F], fp16, tag="X")
                            nc.vector.tensor_scalar(
                                out=x[:], in0=A[k + 2][:],
                                scalar1=c_sb[:, k:k + 1], scalar2=None,
                                op0=OP.add,
                            )
                            nc.vector.tensor_tensor(
                                ak[:], x[:], tmp[:],
                                OP.add if op_add else OP.subtract,
                            )
                        else:  # stt
                            nc.vector.scalar_tensor_tensor(
                                out=ak[:], in0=A[k + 2][:],
                                scalar=c_sb[:, k:k + 1], in1=tmp[:],
                                op0=OP.add,
                                op1=OP.add if op_add else OP.subtract,
                            )
                    else:
                        tmp = tp.tile([P, F], fp32, tag="tmp")
                        nc.vector.tensor_tensor(tmp[:], U2[:], A[k + 1][:], OP.mult)
                        ak = ab.tile([P, F], fp32, tag="A")
                        nc.vector.scalar_tensor_tensor(
                            out=ak[:], in0=A[k + 2][:], scalar=c_sb[:, k:k + 1],
                            in1=tmp[:], op0=OP.add,
                            op1=OP.add if op_add else OP.subtract,
                        )
                    A[k] = ak
                # final: p = c_0 + u*b_1 - b_2, split into column halves so the
                # first half's store overlaps the second half's compute
                U1 = cp.tile([P, F], fp32, tag="U1")
                nc.vector.tensor_scalar(
                    out=U1[:], in0=T[:], scalar1=float(eps[1] / tmax), scalar2=None,
                    op0=OP.mult,
                )
                outt = cp.tile([P, F], fp32, tag="outt")
                H = F // 2
                for hcol in range(2):
                    sl = slice(hcol * H, (hcol + 1) * H)
                    tmpf = tp.tile([P, H], fp32, tag="tmp")
                    nc.vector.tensor_tensor(tmpf[:], U1[:, sl], A[1][:, sl], OP.mult)
                    g = wp.tile([P, H], fp32, tag="g")
                    nc.vector.scalar_tensor_tensor(
                        out=g[:], in0=A[2][:, sl], scalar=c_sb[:, 0:1],
                        in1=tmpf[:], op0=OP.add, op1=OP.add,
                    )
                    nc.vector.tensor_tensor(outt[:, sl], s_all[:, sl], g[:], OP.mult)
                    nc.sync.dma_start(out=y_d[:, sl], in_=outt[:, sl])
            else:
                S0 = cp.tile([P, F], fp32, tag="S0")
                S1 = cp.tile([P, F], fp32, tag="S1")
                qb = cp.tile([P, K], fp32, tag="qb")
                nc.vector.tensor_copy(qb[:], q_ps[:])
                for j in range(F):
                    E = wp.tile([P, K], fp32, tag="E")
                    nc.scalar.activation(
                        E[:], qb[:], AF.Exp, scale=s_all[:, j:j + 1],
                        accum_out=S0[:, j:j + 1],
                    )
                    scr = wp.tile([P, K], fp32, tag="scr")
                    nc.vector.scalar_tensor_tensor(
                        out=scr[:], in0=E[:], scalar=1.0, in1=qb[:],
                        op0=OP.mult, op1=OP.mult, accum_out=S1[:, j:j + 1],
                    )
                rec = cp.tile([P, F], fp32, tag="rec")
                nc.vector.reciprocal(rec[:], S0[:])
                ctx = cp.tile([P, F], fp32, tag="ctx")
                nc.vector.tensor_tensor(ctx[:], S1[:], rec[:], OP.mult)
                th = cp.tile([P, F], fp32, tag="th")
                nc.scalar.activation(th[:], ctx[:], AF.Tanh, scale=0.5)
                gate = cp.tile([P, F], fp32, tag="gate")
                nc.vector.tensor_scalar(
                    out=gate[:], in0=th[:], scalar1=0.5, scalar2=0.5,
                    op0=OP.mult, op1=OP.add,
                )
                outt = cp.tile([P, F], fp32, tag="outt")
                nc.vector.tensor_tensor(outt[:], s_all[:], gate[:], OP.mult)
                nc.sync.dma_start(out=y_d[:], in_=outt[:])

    nc.compile()
    return nc


def _get_nc(variant):
    if variant not in _NC_CACHE:
        _NC_CACHE[variant] = _build_nc(variant)
    return _NC_CACHE[variant]


def _in_maps(key, query):
    _tmax, sn, gt, _eps = _host_constants()
    s2 = key.reshape(B, J)
    h = J // 2
    maps = []
    for c in range(NCORES):
        b, half = divmod(c, 2)
        q = query[b].astype(np.float32)
        qhi = q.astype(np.float16)
        qlo = (q - qhi.astype(np.float32)).astype(np.float16)
        maps.append({
            "s": np.ascontiguousarray(s2[b, half * h:(half + 1) * h].reshape(P, F)),
            "qpair": np.ascontiguousarray(np.stack([qhi, qlo], 0)),
            "sn": sn,
            "gt": gt,
        })
    return maps


def kernel(key, query, _variant=None, _trace=False):
    key = np.ascontiguousarray(key, dtype=np.float32)
    query = np.ascontiguousarray(query, dtype=np.float32)
    variant = _variant or os.environ.get("BASS_KERNEL_VARIANT", "fast")
    nc = _get_nc(variant)
    from concourse.bass_utils import run_bass_kernel_spmd

    res = run_bass_kernel_spmd(
        nc, _in_maps(key, query), list(range(NCORES)), trace=_trace
    )
    h = J // 2
    out = np.empty((B, J), np.float32)
    for c in range(NCORES):
        b, half = divmod(c, 2)
        out[b, half * h:(half + 1) * h] = res.results[c]["y"].reshape(h)
    if _trace:
        kernel.last_results = res
    return out.reshape(key.shape)
